# revision 15
# baseline (speedup 1.0000x reference)
"""Multi-head self-attention (no mask) on 8 TRN2 NeuronCores.

Sharding: tensor-parallel over heads (2 heads/core) for QKV + attention,
then an AllToAll re-shards to row-parallel for the output projection.

v2 structure (fused schedule, all inputs pre-cast to bf16 on host):
  Stage 1: QKV projections for row-chunks 0-3 (batch 0).
  Stage 2: projections for chunks 4-7 (batch 1) interleaved with
     attention chunks (h0, b0, *) so the PE covers ACT's exp latency.
  Stage 3: remaining 12 attention chunks; AllToAll(h0) fires 1/3 in;
     out-projection even-k-slab pre-runs (stashed to SBUF bf16 partials)
     fill the AllToAll(h1) window.
  Stage 4: odd-k-slab accumulation + partial add + bias, stream out.

Attention chunk: scores into a [128,2048] 4-bank PSUM tile (4 MMs), one
wide exp ACTIVATE per quad; PV accumulates v^T expT; softmax denominators
via DVE pairwise tree + a ones[128x128] matmul that broadcasts the sums
to all partitions in one shot; reciprocal_approx_fast + one DVE mul
normalizes. No max-subtraction (scores are O(5)).
"""

import numpy as np

import concourse.bass as bass
import concourse.tile as tile
from concourse import bacc, mybir
from concourse.bass_utils import run_bass_kernel_spmd

F32 = mybir.dt.float32
BF16 = mybir.dt.bfloat16

B, S, H = 2, 2048, 2048
NH, HD = 16, 128
NC = 8
BS = B * S          # 4096 rows total
FL = H // NC        # 256 features per core (2 heads)
HL = NH // NC       # 2 heads per core
RPC = BS // NC      # 512 output rows per core
K16 = H // 128      # 16 contraction tiles
CW = 512            # row-chunk width
QC = 512            # attention q-chunk width
SCALE = 1.0 / float(np.sqrt(HD))

_CACHED = None


def _build():
    nc = bacc.Bacc("TRN2", target_bir_lowering=False, debug=False, num_devices=NC)

    xT_d = nc.dram_tensor("xT_t", [128, K16, BS], BF16, kind="ExternalInput")
    wqT_d = nc.dram_tensor("wqT_t", [128, K16, FL], BF16, kind="ExternalInput")
    wkT_d = nc.dram_tensor("wkT_t", [128, K16, FL], BF16, kind="ExternalInput")
    wvT_d = nc.dram_tensor("wvT_t", [128, K16, FL], BF16, kind="ExternalInput")
    bq_d = nc.dram_tensor("bq", [128, HL], F32, kind="ExternalInput")
    bk_d = nc.dram_tensor("bk", [128, HL], F32, kind="ExternalInput")
    bv_d = nc.dram_tensor("bv_bc", [128, FL], F32, kind="ExternalInput")
    woT_d = nc.dram_tensor("woT_t", [128, K16, H], BF16, kind="ExternalInput")
    bo_d = nc.dram_tensor("bo_bc", [128, H], BF16, kind="ExternalInput")
    onesb_d = nc.dram_tensor("ones_bf", [128, 128], BF16, kind="ExternalInput")
    out_d = nc.dram_tensor("out", [RPC, H], F32, kind="ExternalOutput")

    with tile.TileContext(nc) as tc:
        with (
            tc.tile_pool(name="consts", bufs=1) as cstp,
            tc.tile_pool(name="dram", bufs=1, space="DRAM") as dp,
            tc.tile_pool(name="qkv", bufs=1) as qkvp,
            tc.tile_pool(name="wo01", bufs=1) as wop,
            tc.tile_pool(name="attn", bufs=1) as ap_,
            tc.tile_pool(name="psum", bufs=1, space="PSUM") as pp,
        ):
            ones_bf = cstp.tile([128, 128], BF16)
            bq_sb = cstp.tile([128, HL], F32)
            bk_sb = cstp.tile([128, HL], F32)
            bv_sb = cstp.tile([128, FL], F32)
            bo_sb = cstp.tile([128, H], BF16)

            a2a_in = [dp.tile([NC, 128, RPC], BF16, name=f"a2a_in{h}") for h in range(HL)]
            a2a_out = [dp.tile([NC, 128, RPC], BF16, name=f"a2a_out{h}") for h in range(HL)]

            qT_sb = qkvp.tile([128, HL * BS], BF16)
            kT_sb = qkvp.tile([128, HL * BS], BF16)
            v_sb = qkvp.tile([128, (BS // 128) * FL], BF16)

            won_tiles = {}

            def load_wo(pool, n):
                won = pool.tile([128, K16 * 512], BF16, tag="won", bufs=2)
                nc.sync.dma_start(won[:], woT_d.ap()[:, :, n * 512:(n + 1) * 512])
                won_tiles[n] = won

            # ---------------- projection building blocks ----------------
            def emit_proj_qk(w_sb, b_sb, dst, c, m):
                """One [128 feats x 512 rows] output block of q/k for chunk c."""
                xc = x_tiles[c]
                ps = pp.tile([128, CW], F32, tag="pa", bufs=3)
                for k in range(K16):
                    nc.tensor.matmul(
                        ps[:],
                        w_sb[:, k * FL + m * 128: k * FL + (m + 1) * 128],
                        xc[:, k * CW:(k + 1) * CW],
                        start=(k == 0),
                        stop=(k == K16 - 1),
                    )
                nc.vector.tensor_scalar_add(
                    dst[:, m * BS + c * CW: m * BS + (c + 1) * CW],
                    ps[:],
                    b_sb[:, m:m + 1],
                )

            def emit_proj_v(c, m2):
                """One [128 rows x 256 feats] block of v for chunk c."""
                xc = x_tiles[c]
                ps = pp.tile([128, CW], F32, tag="pa", bufs=3)
                for k in range(K16):
                    nc.tensor.matmul(
                        ps[:, :FL],
                        xc[:, k * CW + m2 * 128: k * CW + (m2 + 1) * 128],
                        wv_sb[:, k * FL:(k + 1) * FL],
                        start=(k == 0),
                        stop=(k == K16 - 1),
                    )
                i = c * (CW // 128) + m2
                nc.vector.tensor_add(
                    v_sb[:, i * FL:(i + 1) * FL], ps[:, :FL], bv_sb[:]
                )

            # ---------------- attention building blocks ----------------
            pend = {}

            def emit_scores_pair(key, h, b, qc, pair):
                """2 score MMs into a 2-bank PSUM tile + one wide exp.
                bufs=2 so the next pair's MMs overlap this pair's exp."""
                base = h * BS + b * S
                if pair == 0:
                    pend[key] = ap_.tile(
                        [128, K16 * QC], BF16, tag="expT", bufs=2, name="expT"
                    )
                expT = pend[key]
                pss = pp.tile([128, 1024], F32, tag="pss", bufs=2)
                for j in range(2):
                    km = pair * 2 + j
                    nc.tensor.matmul(
                        pss[:, j * QC:(j + 1) * QC],
                        kT_sb[:, base + km * 128: base + (km + 1) * 128],
                        qT_sb[:, base + qc * QC: base + (qc + 1) * QC],
                        start=True,
                        stop=True,
                    )
                nc.scalar.activation(
                    expT[:, pair * 2 * QC:(pair + 1) * 2 * QC],
                    pss[:],
                    mybir.ActivationFunctionType.Exp,
                    scale=SCALE,
                )

            def emit_scores_quad(key, h, b, qc, quad):
                emit_scores_pair(key, h, b, qc, 2 * quad)
                emit_scores_pair(key, h, b, qc, 2 * quad + 1)

            psa_pend = {}

            def emit_pv_half(key, h, b, half):
                """Half of the PV accumulation (8 of 16 k-tiles)."""
                expT = pend[key]
                if half == 0:
                    psa_pend[key] = pp.tile(
                        [128, QC], F32, tag="pa", bufs=3, name="psa"
                    )
                psa = psa_pend[key]
                for j in range(8):
                    km = half * 8 + j
                    nc.tensor.matmul(
                        psa[:],
                        v_sb[:, (16 * b + km) * FL + h * 128:
                             (16 * b + km) * FL + (h + 1) * 128],
                        expT[:, km * QC:(km + 1) * QC],
                        start=(km == 0),
                        stop=(km == K16 - 1),
                    )

            def emit_norm(key, h, b, qc):
                """Denominator tree, normalize, ship to the a2a buffer."""
                dest = b * (S // QC) + qc
                expT = pend.pop(key)
                psa = psa_pend.pop(key)
                s2 = ap_.tile([128, 4 * QC], BF16, tag="s2", bufs=1)
                nc.vector.tensor_add(s2[:], expT[:, :4 * QC], expT[:, 4 * QC:8 * QC])
                nc.vector.tensor_add(s2[:], s2[:], expT[:, 8 * QC:12 * QC])
                nc.vector.tensor_add(s2[:], s2[:], expT[:, 12 * QC:])
                s3 = ap_.tile([128, 2 * QC], BF16, tag="s3", bufs=1)
                nc.vector.tensor_add(s3[:], s2[:, :2 * QC], s2[:, 2 * QC:])
                s4 = ap_.tile([128, QC], BF16, tag="s4", bufs=2)
                nc.vector.tensor_add(s4[:], s3[:, :QC], s3[:, QC:])
                # broadcast column sums to all 128 partitions in one MM
                psum_bc = pp.tile([128, QC], F32, tag="pbc", bufs=1)
                nc.tensor.matmul(psum_bc[:], ones_bf[:], s4[:], start=True, stop=True)
                rb = ap_.tile([128, QC], F32, tag="rb", bufs=2)
                nc.vector.reciprocal_approx_fast(rb[:], psum_bc[:])
                att = ap_.tile([128, QC], BF16, tag="att", bufs=2)
                nc.vector.tensor_mul(att[:], psa[:], rb[:])
                nc.gpsimd.dma_start(a2a_in[h][dest, :, :], att[:])

            def emit_pv_norm(key, h, b, qc):
                emit_pv_half(key, h, b, 0)
                emit_pv_half(key, h, b, 1)
                emit_norm(key, h, b, qc)

            # ---------------- stages 1+2 (x/w pools open) ----------------
            with (
                tc.tile_pool(name="wgt", bufs=1) as wp,
                tc.tile_pool(name="xbf", bufs=1) as xbp,
            ):
                wq_sb = wp.tile([128, K16 * FL], BF16, tag="wq")
                wk_sb = wp.tile([128, K16 * FL], BF16, tag="wk")
                wv_sb = wp.tile([128, K16 * FL], BF16, tag="wv")

                x_tiles = {}

                def load_x(c):
                    xc = xbp.tile([128, K16 * CW], BF16, tag="x", bufs=2)
                    nc.sync.dma_start(xc[:], xT_d.ap()[:, :, c * CW:(c + 1) * CW])
                    x_tiles[c] = xc

                # Critical-path-ordered first loads: interleave wq/x0
                # quarters so the first matmuls start as early as possible.
                xc0 = xbp.tile([128, K16 * CW], BF16, tag="x", bufs=2, name="xc")
                x_tiles[0] = xc0
                for p in range(4):
                    nc.sync.dma_start(
                        wq_sb[:, p * 4 * FL:(p + 1) * 4 * FL],
                        wqT_d.ap()[:, p * 4:(p + 1) * 4, :],
                    )
                    nc.sync.dma_start(
                        xc0[:, p * 4 * CW:(p + 1) * 4 * CW],
                        xT_d.ap()[:, p * 4:(p + 1) * 4, :CW],
                    )
                nc.sync.dma_start(bq_sb[:], bq_d.ap()[:])
                nc.sync.dma_start(bk_sb[:], bk_d.ap()[:])
                nc.sync.dma_start(wk_sb[:], wkT_d.ap()[:])
                load_x(1)
                nc.sync.dma_start(wv_sb[:], wvT_d.ap()[:])
                nc.sync.dma_start(bv_sb[:], bv_d.ap()[:])
                nc.sync.dma_start(ones_bf[:], onesb_d.ap()[:])

                # Stage 1: chunks 0-3 (batch 0)
                for c in range(4):
                    if 2 <= c + 1 < 4:
                        load_x(c + 1)
                    for m in range(HL):
                        emit_proj_qk(wq_sb, bq_sb, qT_sb, c, m)
                        emit_proj_qk(wk_sb, bk_sb, kT_sb, c, m)
                    for m2 in range(CW // 128):
                        emit_proj_v(c, m2)

                # Stage 2: chunks 4-7 interleaved with attention (h0, b0, *)
                load_x(4)
                nc.sync.dma_start(bo_sb[:], bo_d.ap()[:])
                for i in range(4):
                    c = 4 + i
                    if c + 1 < 8:
                        load_x(c + 1)
                    if i < 2:
                        load_wo(wop, i)
                    key = (0, 0, i)
                    emit_scores_quad(key, 0, 0, i, 0)
                    emit_proj_qk(wq_sb, bq_sb, qT_sb, c, 0)
                    emit_scores_quad(key, 0, 0, i, 1)
                    emit_proj_qk(wq_sb, bq_sb, qT_sb, c, 1)
                    emit_scores_quad(key, 0, 0, i, 2)
                    emit_proj_qk(wk_sb, bk_sb, kT_sb, c, 0)
                    emit_scores_quad(key, 0, 0, i, 3)
                    emit_proj_qk(wk_sb, bk_sb, kT_sb, c, 1)
                    if i < 3:
                        emit_pv_norm(key, 0, 0, i)
                    for m2 in range(CW // 128):
                        emit_proj_v(c, m2)

            # ---------------- stages 3+4 ----------------
            with (
                tc.tile_pool(name="wo23", bufs=1) as wop2,
                tc.tile_pool(name="aTp", bufs=1) as atp,
                tc.tile_pool(name="cpart", bufs=1) as cpp,
                tc.tile_pool(name="outC", bufs=1) as ocp,
            ):
                aT = atp.tile([128, K16 * RPC], BF16)
                partials = cpp.tile([128, 16 * 512], BF16)
                ctiles = [(n, m) for n in range(4) for m in range(4)]

                def emit_c_even(t):
                    n, m = ctiles[t]
                    won = won_tiles[n]
                    pso = pp.tile([128, 512], F32, tag="pa", bufs=3)
                    for j in range(8):
                        k = 2 * j
                        nc.tensor.matmul(
                            pso[:],
                            aT[:, k * RPC + m * 128: k * RPC + (m + 1) * 128],
                            won[:, k * 512:(k + 1) * 512],
                            start=(j == 0),
                            stop=(j == 7),
                        )
                    # stash evens + bias as a bf16 partial
                    nc.vector.tensor_add(
                        partials[:, t * 512:(t + 1) * 512],
                        pso[:],
                        bo_sb[:, n * 512:(n + 1) * 512],
                    )

                def emit_c_odd(t):
                    n, m = ctiles[t]
                    won = won_tiles[n]
                    pso = pp.tile([128, 512], F32, tag="pa", bufs=3)
                    for j in range(8):
                        k = 2 * j + 1
                        nc.tensor.matmul(
                            pso[:],
                            aT[:, k * RPC + m * 128: k * RPC + (m + 1) * 128],
                            won[:, k * 512:(k + 1) * 512],
                            start=(j == 0),
                            stop=(j == 7),
                        )
                    ot = ocp.tile([128, 512], F32, tag="ot", bufs=3)
                    nc.vector.tensor_add(
                        ot[:], pso[:], partials[:, t * 512:(t + 1) * 512]
                    )
                    nc.sync.dma_start(
                        out_d.ap()[m * 128:(m + 1) * 128, n * 512:(n + 1) * 512],
                        ot[:],
                    )

                # Stage 3: 1-deep software pipeline — chunk i+1's score
                # quads are woven between chunk i's PV halves so the PE
                # never waits on ACT.  Order: (h0,b1,*) -> A2A(h0);
                # (h1,b0,*), (h1,b1,*) -> A2A(h1).  A few C-even pre-runs
                # interleave late; the bulk fills the A2A(h1) window.
                s3_chunks = [(0, 1, qc) for qc in range(4)] + [
                    (1, b, qc) for b in range(B) for qc in range(4)
                ]

                def fire_a2a(h):
                    nc.gpsimd.collective_compute(
                        "AllToAll",
                        mybir.AluOpType.bypass,
                        ins=[a2a_in[h].opt()],
                        outs=[a2a_out[h].opt()],
                        replica_groups=[list(range(NC))],
                    )
                    for g in range(h, K16, 2):
                        nc.sync.dma_start(
                            aT[:, g * RPC:(g + 1) * RPC], a2a_out[h][g // 2, :, :]
                        )

                prev = (0, 0, 3)
                for idx, (h, b, qc) in enumerate(s3_chunks):
                    if idx < 2:
                        load_wo(wop2, 2 + idx)
                    key = (h, b, qc)
                    emit_scores_quad(key, h, b, qc, 0)
                    emit_pv_half(prev, prev[0], prev[1], 0)
                    emit_scores_quad(key, h, b, qc, 1)
                    emit_pv_half(prev, prev[0], prev[1], 1)
                    emit_scores_quad(key, h, b, qc, 2)
                    emit_norm(prev, prev[0], prev[1], prev[2])
                    if prev == (0, 1, 3):
                        fire_a2a(0)
                    emit_scores_quad(key, h, b, qc, 3)
                    prev = key
                emit_pv_norm(prev, prev[0], prev[1], prev[2])
                fire_a2a(1)
                for t in range(16):
                    emit_c_even(t)

                # Stage 4: odd halves + combine
                for t in range(16):
                    emit_c_odd(t)

    nc.compile()
    return nc


def _get_nc():
    global _CACHED
    if _CACHED is None:
        _CACHED = _build()
    return _CACHED


def _prep_in_maps(x, Wq, bq, Wk, bk, Wv, bv, Wo, bo):
    import ml_dtypes

    bf = ml_dtypes.bfloat16

    def tile_kmaj(a2d):
        # [H, N] -> [128, K16, N] with row r = k*128 + p
        h, n = a2d.shape
        return np.ascontiguousarray(
            a2d.reshape(K16, 128, n).transpose(1, 0, 2).astype(bf)
        )

    xT_t = tile_kmaj(x.reshape(BS, H).T)
    woT_t = tile_kmaj(Wo.T)
    bo_bc = np.ascontiguousarray(np.broadcast_to(bo, (128, H)).astype(bf))
    ones_bf = np.ones((128, 128), bf)
    in_maps = []
    for c in range(NC):
        sl = slice(FL * c, FL * (c + 1))
        in_maps.append(
            {
                "xT_t": xT_t,
                "wqT_t": tile_kmaj(np.ascontiguousarray(Wq[sl, :].T)),
                "wkT_t": tile_kmaj(np.ascontiguousarray(Wk[sl, :].T)),
                "wvT_t": tile_kmaj(np.ascontiguousarray(Wv[sl, :].T)),
                "bq": np.ascontiguousarray(bq[sl].reshape(HL, 128).T),
                "bk": np.ascontiguousarray(bk[sl].reshape(HL, 128).T),
                "bv_bc": np.ascontiguousarray(np.broadcast_to(bv[sl], (128, FL))),
                "woT_t": woT_t,
                "bo_bc": bo_bc,
                "ones_bf": ones_bf,
            }
        )
    return in_maps


def run(in_maps, trace=False):
    nc = _get_nc()
    return run_bass_kernel_spmd(nc, in_maps, core_ids=list(range(NC)), trace=trace)


def kernel(x, Wq, bq, Wk, bk, Wv, bv, Wo, bo):
    args = [np.asarray(a, dtype=np.float32) for a in (x, Wq, bq, Wk, bk, Wv, bv, Wo, bo)]
    in_maps = _prep_in_maps(*args)
    res = run(in_maps)
    out = np.concatenate([res.results[c]["out"] for c in range(NC)], axis=0)
    return out.reshape(B, S, H)


# revision 16
# speedup vs baseline: 1.1842x; 1.1842x over previous
"""Multi-head self-attention (no mask) on 8 TRN2 NeuronCores.

Sharding: tensor-parallel over heads (2 heads/core) for QKV + attention,
then an AllToAll re-shards to row-parallel for the output projection.

v2 structure (fused schedule, all inputs pre-cast to bf16 on host):
  Stage 1: QKV projections for row-chunks 0-3 (batch 0).
  Stage 2: projections for chunks 4-7 (batch 1) interleaved with
     attention chunks (h0, b0, *) so the PE covers ACT's exp latency.
  Stage 3: remaining 12 attention chunks; AllToAll(h0) fires 1/3 in;
     out-projection even-k-slab pre-runs (stashed to SBUF bf16 partials)
     fill the AllToAll(h1) window.
  Stage 4: odd-k-slab accumulation + partial add + bias, stream out.

Attention chunk: scores into a [128,2048] 4-bank PSUM tile (4 MMs), one
wide exp ACTIVATE per quad; PV accumulates v^T expT; softmax denominators
via DVE pairwise tree + a ones[128x128] matmul that broadcasts the sums
to all partitions in one shot; reciprocal_approx_fast + one DVE mul
normalizes. No max-subtraction (scores are O(5)).
"""

import numpy as np

import concourse.bass as bass
import concourse.tile as tile
from concourse import bacc, mybir
from concourse.bass_utils import run_bass_kernel_spmd

F32 = mybir.dt.float32
BF16 = mybir.dt.bfloat16

B, S, H = 2, 2048, 2048
NH, HD = 16, 128
NC = 8
BS = B * S          # 4096 rows total
FL = H // NC        # 256 features per core (2 heads)
HL = NH // NC       # 2 heads per core
RPC = BS // NC      # 512 output rows per core
K16 = H // 128      # 16 contraction tiles
CW = 512            # row-chunk width
QC = 512            # attention q-chunk width
SCALE = 1.0 / float(np.sqrt(HD))

_CACHED = None


def _build():
    nc = bacc.Bacc("TRN2", target_bir_lowering=False, debug=False, num_devices=NC)

    xT_d = nc.dram_tensor("xT_t", [128, K16, BS], BF16, kind="ExternalInput")
    wqT_d = nc.dram_tensor("wqT_t", [128, K16, FL], BF16, kind="ExternalInput")
    wkT_d = nc.dram_tensor("wkT_t", [128, K16, FL], BF16, kind="ExternalInput")
    wvT_d = nc.dram_tensor("wvT_t", [128, K16, FL], BF16, kind="ExternalInput")
    bq_d = nc.dram_tensor("bq", [128, HL], F32, kind="ExternalInput")
    bk_d = nc.dram_tensor("bk", [128, HL], F32, kind="ExternalInput")
    bv_d = nc.dram_tensor("bv_bc", [128, FL], F32, kind="ExternalInput")
    woT_d = nc.dram_tensor("woT_t", [128, K16, H], BF16, kind="ExternalInput")
    bo_d = nc.dram_tensor("bo_bc", [128, H], BF16, kind="ExternalInput")
    onesb_d = nc.dram_tensor("ones_bf", [128, 128], BF16, kind="ExternalInput")
    out_d = nc.dram_tensor("out", [RPC, H], F32, kind="ExternalOutput")

    with tile.TileContext(nc) as tc:
        with (
            tc.tile_pool(name="consts", bufs=1) as cstp,
            tc.tile_pool(name="dram", bufs=1, space="DRAM") as dp,
            tc.tile_pool(name="qkv", bufs=1) as qkvp,
            tc.tile_pool(name="wo01", bufs=1) as wop,
            tc.tile_pool(name="attn", bufs=1) as ap_,
            tc.tile_pool(name="psum", bufs=1, space="PSUM") as pp,
        ):
            ones_bf = cstp.tile([128, 128], BF16)
            bq_sb = cstp.tile([128, HL], F32)
            bk_sb = cstp.tile([128, HL], F32)
            bv_sb = cstp.tile([128, FL], F32)
            bo_sb = cstp.tile([128, H], BF16)

            a2a_in = [dp.tile([NC, 128, RPC], BF16, name=f"a2a_in{h}") for h in range(HL)]
            a2a_out = [dp.tile([NC, 128, RPC], BF16, name=f"a2a_out{h}") for h in range(HL)]

            qT_sb = qkvp.tile([128, HL * BS], BF16)
            kT_sb = qkvp.tile([128, HL * BS], BF16)
            v_sb = qkvp.tile([128, (BS // 128) * FL], BF16)

            won_tiles = {}

            def load_wo(pool, n):
                won = pool.tile([128, K16 * 512], BF16, tag="won", bufs=2)
                nc.sync.dma_start(won[:], woT_d.ap()[:, :, n * 512:(n + 1) * 512])
                won_tiles[n] = won

            # ---------------- projection building blocks ----------------
            def emit_proj_qk(w_sb, b_sb, dst, c, m):
                """One [128 feats x 512 rows] output block of q/k for chunk c."""
                xc = x_tiles[c]
                ps = pp.tile([128, CW], F32, tag="pa", bufs=3)
                for k in range(K16):
                    nc.tensor.matmul(
                        ps[:],
                        w_sb[:, k * FL + m * 128: k * FL + (m + 1) * 128],
                        xc[:, k * CW:(k + 1) * CW],
                        start=(k == 0),
                        stop=(k == K16 - 1),
                    )
                nc.vector.tensor_scalar_add(
                    dst[:, m * BS + c * CW: m * BS + (c + 1) * CW],
                    ps[:],
                    b_sb[:, m:m + 1],
                )

            def emit_proj_v(c, m2):
                """One [128 rows x 256 feats] block of v for chunk c."""
                xc = x_tiles[c]
                ps = pp.tile([128, CW], F32, tag="pa", bufs=3)
                for k in range(K16):
                    nc.tensor.matmul(
                        ps[:, :FL],
                        xc[:, k * CW + m2 * 128: k * CW + (m2 + 1) * 128],
                        wv_sb[:, k * FL:(k + 1) * FL],
                        start=(k == 0),
                        stop=(k == K16 - 1),
                    )
                i = c * (CW // 128) + m2
                nc.vector.tensor_add(
                    v_sb[:, i * FL:(i + 1) * FL], ps[:, :FL], bv_sb[:]
                )

            # ---------------- attention building blocks ----------------
            pend = {}

            def emit_scores_pair(key, h, b, qc, pair):
                """2 score MMs into a 2-bank PSUM tile + one wide exp.
                bufs=2 so the next pair's MMs overlap this pair's exp."""
                base = h * BS + b * S
                if pair == 0:
                    pend[key] = ap_.tile(
                        [128, K16 * QC], BF16, tag="expT", bufs=2, name="expT"
                    )
                expT = pend[key]
                pss = pp.tile([128, 1024], F32, tag="pss", bufs=2)
                for j in range(2):
                    km = pair * 2 + j
                    nc.tensor.matmul(
                        pss[:, j * QC:(j + 1) * QC],
                        kT_sb[:, base + km * 128: base + (km + 1) * 128],
                        qT_sb[:, base + qc * QC: base + (qc + 1) * QC],
                        start=True,
                        stop=True,
                    )
                nc.scalar.activation(
                    expT[:, pair * 2 * QC:(pair + 1) * 2 * QC],
                    pss[:],
                    mybir.ActivationFunctionType.Exp,
                    scale=SCALE,
                )

            def emit_scores_quad(key, h, b, qc, quad):
                emit_scores_pair(key, h, b, qc, 2 * quad)
                emit_scores_pair(key, h, b, qc, 2 * quad + 1)

            psa_pend = {}

            def emit_pv_half(key, h, b, half):
                """Half of the PV accumulation (8 of 16 k-tiles)."""
                expT = pend[key]
                if half == 0:
                    psa_pend[key] = pp.tile(
                        [128, QC], F32, tag="pa", bufs=3, name="psa"
                    )
                psa = psa_pend[key]
                for j in range(8):
                    km = half * 8 + j
                    nc.tensor.matmul(
                        psa[:],
                        v_sb[:, (16 * b + km) * FL + h * 128:
                             (16 * b + km) * FL + (h + 1) * 128],
                        expT[:, km * QC:(km + 1) * QC],
                        start=(km == 0),
                        stop=(km == K16 - 1),
                    )

            def emit_norm(key, h, b, qc):
                """Denominator tree, normalize, ship to the a2a buffer."""
                dest = b * (S // QC) + qc
                expT = pend.pop(key)
                psa = psa_pend.pop(key)
                s2 = ap_.tile([128, 4 * QC], BF16, tag="s2", bufs=1)
                nc.vector.tensor_add(s2[:], expT[:, :4 * QC], expT[:, 4 * QC:8 * QC])
                nc.vector.tensor_add(s2[:], s2[:], expT[:, 8 * QC:12 * QC])
                nc.vector.tensor_add(s2[:], s2[:], expT[:, 12 * QC:])
                s3 = ap_.tile([128, 2 * QC], BF16, tag="s3", bufs=1)
                nc.vector.tensor_add(s3[:], s2[:, :2 * QC], s2[:, 2 * QC:])
                s4 = ap_.tile([128, QC], BF16, tag="s4", bufs=2)
                nc.vector.tensor_add(s4[:], s3[:, :QC], s3[:, QC:])
                # broadcast column sums to all 128 partitions in one MM
                psum_bc = pp.tile([128, QC], F32, tag="pa", bufs=3, name="psum_bc")
                nc.tensor.matmul(psum_bc[:], ones_bf[:], s4[:], start=True, stop=True)
                rb = ap_.tile([128, QC], F32, tag="rb", bufs=2)
                nc.vector.reciprocal_approx_fast(rb[:], psum_bc[:])
                att = ap_.tile([128, QC], BF16, tag="att", bufs=2)
                nc.vector.tensor_mul(att[:], psa[:], rb[:])
                nc.gpsimd.dma_start(a2a_in[h][dest, :, :], att[:])

            def emit_pv_norm(key, h, b, qc):
                emit_pv_half(key, h, b, 0)
                emit_pv_half(key, h, b, 1)
                emit_norm(key, h, b, qc)

            # ---------------- stages 1+2 (x/w pools open) ----------------
            with (
                tc.tile_pool(name="wgt", bufs=1) as wp,
                tc.tile_pool(name="xbf", bufs=1) as xbp,
            ):
                wq_sb = wp.tile([128, K16 * FL], BF16, tag="wq")
                wk_sb = wp.tile([128, K16 * FL], BF16, tag="wk")
                wv_sb = wp.tile([128, K16 * FL], BF16, tag="wv")

                x_tiles = {}

                def load_x(c):
                    xc = xbp.tile([128, K16 * CW], BF16, tag="x", bufs=2)
                    nc.sync.dma_start(xc[:], xT_d.ap()[:, :, c * CW:(c + 1) * CW])
                    x_tiles[c] = xc

                # Critical-path-ordered first loads: interleave wq/x0
                # quarters so the first matmuls start as early as possible.
                xc0 = xbp.tile([128, K16 * CW], BF16, tag="x", bufs=2, name="xc")
                x_tiles[0] = xc0
                for p in range(4):
                    nc.sync.dma_start(
                        wq_sb[:, p * 4 * FL:(p + 1) * 4 * FL],
                        wqT_d.ap()[:, p * 4:(p + 1) * 4, :],
                    )
                    nc.sync.dma_start(
                        xc0[:, p * 4 * CW:(p + 1) * 4 * CW],
                        xT_d.ap()[:, p * 4:(p + 1) * 4, :CW],
                    )
                nc.sync.dma_start(bq_sb[:], bq_d.ap()[:])
                nc.sync.dma_start(bk_sb[:], bk_d.ap()[:])
                nc.sync.dma_start(wk_sb[:], wkT_d.ap()[:])
                load_x(1)
                nc.sync.dma_start(wv_sb[:], wvT_d.ap()[:])
                nc.sync.dma_start(bv_sb[:], bv_d.ap()[:])
                nc.sync.dma_start(ones_bf[:], onesb_d.ap()[:])

                # Stage 1: chunks 0-3 (batch 0)
                for c in range(4):
                    if 2 <= c + 1 < 4:
                        load_x(c + 1)
                    for m in range(HL):
                        emit_proj_qk(wq_sb, bq_sb, qT_sb, c, m)
                        emit_proj_qk(wk_sb, bk_sb, kT_sb, c, m)
                    for m2 in range(CW // 128):
                        emit_proj_v(c, m2)

                # Stage 2: chunks 4-7 interleaved with attention (h0, b0, *)
                load_x(4)
                nc.sync.dma_start(bo_sb[:], bo_d.ap()[:])
                for i in range(4):
                    c = 4 + i
                    if c + 1 < 8:
                        load_x(c + 1)
                    if i < 2:
                        load_wo(wop, i)
                    key = (0, 0, i)
                    emit_scores_quad(key, 0, 0, i, 0)
                    emit_proj_qk(wq_sb, bq_sb, qT_sb, c, 0)
                    emit_scores_quad(key, 0, 0, i, 1)
                    emit_proj_qk(wq_sb, bq_sb, qT_sb, c, 1)
                    emit_scores_quad(key, 0, 0, i, 2)
                    emit_proj_qk(wk_sb, bk_sb, kT_sb, c, 0)
                    emit_scores_quad(key, 0, 0, i, 3)
                    emit_proj_qk(wk_sb, bk_sb, kT_sb, c, 1)
                    if i < 3:
                        emit_pv_norm(key, 0, 0, i)
                    for m2 in range(CW // 128):
                        emit_proj_v(c, m2)

            # ---------------- stages 3+4 ----------------
            with (
                tc.tile_pool(name="wo23", bufs=1) as wop2,
                tc.tile_pool(name="aTp", bufs=1) as atp,
                tc.tile_pool(name="cpart", bufs=1) as cpp,
                tc.tile_pool(name="outC", bufs=1) as ocp,
            ):
                aT = atp.tile([128, K16 * RPC], BF16)
                partials = cpp.tile([128, 16 * 512], BF16)
                ctiles = [(n, m) for n in range(4) for m in range(4)]

                def emit_c_even(t):
                    n, m = ctiles[t]
                    won = won_tiles[n]
                    pso = pp.tile([128, 512], F32, tag="pa", bufs=3)
                    for j in range(8):
                        k = 2 * j
                        nc.tensor.matmul(
                            pso[:],
                            aT[:, k * RPC + m * 128: k * RPC + (m + 1) * 128],
                            won[:, k * 512:(k + 1) * 512],
                            start=(j == 0),
                            stop=(j == 7),
                        )
                    # stash evens + bias as a bf16 partial
                    nc.vector.tensor_add(
                        partials[:, t * 512:(t + 1) * 512],
                        pso[:],
                        bo_sb[:, n * 512:(n + 1) * 512],
                    )

                def emit_c_odd(t):
                    n, m = ctiles[t]
                    won = won_tiles[n]
                    pso = pp.tile([128, 512], F32, tag="pa", bufs=3)
                    for j in range(8):
                        k = 2 * j + 1
                        nc.tensor.matmul(
                            pso[:],
                            aT[:, k * RPC + m * 128: k * RPC + (m + 1) * 128],
                            won[:, k * 512:(k + 1) * 512],
                            start=(j == 0),
                            stop=(j == 7),
                        )
                    ot = ocp.tile([128, 512], F32, tag="ot", bufs=3)
                    nc.vector.tensor_add(
                        ot[:], pso[:], partials[:, t * 512:(t + 1) * 512]
                    )
                    nc.sync.dma_start(
                        out_d.ap()[m * 128:(m + 1) * 128, n * 512:(n + 1) * 512],
                        ot[:],
                    )

                # Stage 3: 1-deep software pipeline — chunk i+1's score
                # quads are woven between chunk i's PV halves so the PE
                # never waits on ACT.  Order: (h0,b1,*) -> A2A(h0);
                # (h1,b0,*), (h1,b1,*) -> A2A(h1).  A few C-even pre-runs
                # interleave late; the bulk fills the A2A(h1) window.
                s3_chunks = [(0, 1, qc) for qc in range(4)] + [
                    (1, b, qc) for b in range(B) for qc in range(4)
                ]

                def fire_a2a(h):
                    nc.gpsimd.collective_compute(
                        "AllToAll",
                        mybir.AluOpType.bypass,
                        ins=[a2a_in[h].opt()],
                        outs=[a2a_out[h].opt()],
                        replica_groups=[list(range(NC))],
                    )
                    for g in range(h, K16, 2):
                        nc.sync.dma_start(
                            aT[:, g * RPC:(g + 1) * RPC], a2a_out[h][g // 2, :, :]
                        )

                prev = (0, 0, 3)
                for idx, (h, b, qc) in enumerate(s3_chunks):
                    if idx < 2:
                        load_wo(wop2, 2 + idx)
                    key = (h, b, qc)
                    emit_scores_quad(key, h, b, qc, 0)
                    emit_pv_half(prev, prev[0], prev[1], 0)
                    emit_scores_quad(key, h, b, qc, 1)
                    emit_pv_half(prev, prev[0], prev[1], 1)
                    emit_scores_quad(key, h, b, qc, 2)
                    emit_norm(prev, prev[0], prev[1], prev[2])
                    if prev == (0, 1, 3):
                        fire_a2a(0)
                    emit_scores_quad(key, h, b, qc, 3)
                    prev = key
                emit_pv_norm(prev, prev[0], prev[1], prev[2])
                fire_a2a(1)
                for t in range(16):
                    emit_c_even(t)

                # Stage 4: odd halves + combine
                for t in range(16):
                    emit_c_odd(t)

    nc.compile()
    return nc


def _get_nc():
    global _CACHED
    if _CACHED is None:
        _CACHED = _build()
    return _CACHED


def _prep_in_maps(x, Wq, bq, Wk, bk, Wv, bv, Wo, bo):
    import ml_dtypes

    bf = ml_dtypes.bfloat16

    def tile_kmaj(a2d):
        # [H, N] -> [128, K16, N] with row r = k*128 + p
        h, n = a2d.shape
        return np.ascontiguousarray(
            a2d.reshape(K16, 128, n).transpose(1, 0, 2).astype(bf)
        )

    xT_t = tile_kmaj(x.reshape(BS, H).T)
    woT_t = tile_kmaj(Wo.T)
    bo_bc = np.ascontiguousarray(np.broadcast_to(bo, (128, H)).astype(bf))
    ones_bf = np.ones((128, 128), bf)
    in_maps = []
    for c in range(NC):
        sl = slice(FL * c, FL * (c + 1))
        in_maps.append(
            {
                "xT_t": xT_t,
                "wqT_t": tile_kmaj(np.ascontiguousarray(Wq[sl, :].T)),
                "wkT_t": tile_kmaj(np.ascontiguousarray(Wk[sl, :].T)),
                "wvT_t": tile_kmaj(np.ascontiguousarray(Wv[sl, :].T)),
                "bq": np.ascontiguousarray(bq[sl].reshape(HL, 128).T),
                "bk": np.ascontiguousarray(bk[sl].reshape(HL, 128).T),
                "bv_bc": np.ascontiguousarray(np.broadcast_to(bv[sl], (128, FL))),
                "woT_t": woT_t,
                "bo_bc": bo_bc,
                "ones_bf": ones_bf,
            }
        )
    return in_maps


def run(in_maps, trace=False):
    nc = _get_nc()
    return run_bass_kernel_spmd(nc, in_maps, core_ids=list(range(NC)), trace=trace)


def kernel(x, Wq, bq, Wk, bk, Wv, bv, Wo, bo):
    args = [np.asarray(a, dtype=np.float32) for a in (x, Wq, bq, Wk, bk, Wv, bv, Wo, bo)]
    in_maps = _prep_in_maps(*args)
    res = run(in_maps)
    out = np.concatenate([res.results[c]["out"] for c in range(NC)], axis=0)
    return out.reshape(B, S, H)
